# revision 11
# baseline (speedup 1.0000x reference)
"""Trainium2 Bass kernel: GarmentPersonCrossAttention (B=4, N=2048, M=1024,
DQ=1024, DC=768, H=16, DH=64), distributed over 8 NeuronCores.

Sharding: core i handles batch i//2 and person-row half i%2 (1024 rows).
Everything is local per core (garment-side work is recomputed by both
cores of a batch pair) -- no collectives.

Host-side algebraic folds (exact linear algebra, numpy):
  - LN affine (gamma) folded into Wq/Wk/Wv; softmax scale into Wq.
  - concat([residual, att]) @ Wf + bf
        = residual @ Wf[:DQ] + att @ (Wo @ Wf[DQ:]) + (bo @ Wf[DQ:] + bf)
  - LayerNorm itself is folded into the projections:
        q[n,i] = rstd[n] * ( (x @ Wq')[n,i] - mu[n]*colsum_q[i] + std[n]*bq[i] )
    so projections run on the RAW transposed inputs plus one K=2
    "correction" matmul (rows: -mu, std), and the per-row rstd is applied
    during PSUM evacuation (DVE multiply against a broadcast row for the
    feature-major q/k, ACT per-partition scale for the row-major v).

Device pipeline per core (bf16 matmuls, fp32 PSUM accumulation):
  - DMA-transpose raw xp/xg into feature-major xpT/xgT (no LN round trip).
  - LN stats (bn_stats) on row-major tiles -> -mu/std/rstd columns;
    DMA-bounced into rows for the correction matmuls + rstd broadcasts.
  - Per head-pair p: kT/qT projection chains (6-8 MMs + corr MM, DVE
    rstd evac), then attention: per mt, two row-tiled concurrent score
    MMs (K=64 at array rows 0-63 / 64-127) into a 2-bank PSUM group,
    ONE 1024-wide exp (ACT), two attV MMs accumulating into a 2-bank pa
    (ones column yields denominators at partition 64).  Softmax
    normalization via reciprocal_approx_fast on [1,1024] + DMA broadcast
    bounce.  Residual-path output chains (xT @ Wf_top) interleave as PE
    filler; att @ (Wo Wf_bot) chains run at the end.
"""

import os
import sys

import numpy as np

for _p in ("/opt/trn_rl_repo",):
    if _p not in sys.path and os.path.isdir(_p):
        sys.path.append(_p)

import ml_dtypes

# Problem constants (hardcoded per contest rules).
B, N, M = 4, 2048, 1024
DQ, DC = 1024, 768
H, DH = 16, 64
INNER = H * DH
SCALE = DH ** -0.5
EPS = 1e-5
NCORES = 8
NPC = N // 2          # person rows per core
P = 128               # partitions
NT = NPC // P         # 8 person row tiles per core
MT = M // P           # 8 garment row tiles
KQ = DQ // P          # 8 contraction tiles for person features
KC = DC // P          # 6 contraction tiles for garment features
KI = INNER // P       # 8 inner tiles
NPAIR = H // 2        # 8 head pairs (pair p = heads 2p, 2p+1 = inner tile p)

_CACHE = {}


def _build_nc():
    import concourse.bass as bass
    import concourse.tile as tile
    from concourse import bacc, mybir
    from contextlib import ExitStack

    f32 = mybir.dt.float32
    bf16 = mybir.dt.bfloat16
    AF = mybir.ActivationFunctionType

    nc = bacc.Bacc("TRN2", target_bir_lowering=False, debug=False)

    # ---- DRAM parameters (per-core shards; weights replicated) ----
    xp = nc.dram_tensor("xp", [NPC, DQ], bf16, kind="ExternalInput").ap()
    xg = nc.dram_tensor("xg", [M, DC], bf16, kind="ExternalInput").ap()
    wq = nc.dram_tensor("wq", [DQ, INNER], bf16, kind="ExternalInput").ap()
    wk = nc.dram_tensor("wk", [DC, INNER], bf16, kind="ExternalInput").ap()
    wv = nc.dram_tensor("wv", [DC, INNER], bf16, kind="ExternalInput").ap()
    wof = nc.dram_tensor("wof", [INNER, DQ], bf16, kind="ExternalInput").ap()
    wft = nc.dram_tensor("wft", [DQ, DQ], bf16, kind="ExternalInput").ap()
    cq = nc.dram_tensor("cq", [2, INNER], bf16, kind="ExternalInput").ap()
    ck = nc.dram_tensor("ck", [2, INNER], bf16, kind="ExternalInput").ap()
    cv = nc.dram_tensor("cv", [2, INNER], bf16, kind="ExternalInput").ap()
    bout = nc.dram_tensor("bout", [1, DQ], bf16, kind="ExternalInput").ap()
    out = nc.dram_tensor("out", [NPC, DQ], f32, kind="ExternalOutput").ap()

    # Internal DRAM scratch.
    # Per row-tile stats [mt, (negmu, std, rstd), 128] for each side.
    statp_d = nc.dram_tensor("statp_scratch", [NT * 3 * P], f32).ap()
    statg_d = nc.dram_tensor("statg_scratch", [MT * 3 * P], f32).ap()
    rb_d = nc.dram_tensor("denom_scratch", [H, NPC], f32).ap()
    rb2_d = nc.dram_tensor("recip_scratch", [H, NPC], f32).ap()

    with tile.TileContext(nc) as tc, ExitStack() as ctx:
        # PSUM: pa (2 banks) + score groups (2x2 banks) + chains (2x1).
        pa_pool = ctx.enter_context(tc.tile_pool(name="pa", bufs=1, space="PSUM"))
        sg_pool = ctx.enter_context(tc.tile_pool(name="sg", bufs=2, space="PSUM"))
        ch_pool = ctx.enter_context(tc.tile_pool(name="chain", bufs=2, space="PSUM"))

        const = ctx.enter_context(tc.tile_pool(name="const", bufs=1, side="left"))
        small = ctx.enter_context(tc.tile_pool(name="small", bufs=4, side="left"))

        # ---- constants ----
        eps_t = const.tile([P, 1], f32, name="eps_t")
        nc.vector.memset(eps_t, EPS)
        ones_row = const.tile([1, P], bf16, name="ones_row")
        nc.vector.memset(ones_row, 1.0)
        cq_t = const.tile([2, INNER], bf16, name="cq_t")
        nc.sync.dma_start(out=cq_t, in_=cq)
        ck_t = const.tile([2, INNER], bf16, name="ck_t")
        nc.sync.dma_start(out=ck_t, in_=ck)
        cv_t = const.tile([2, INNER], bf16, name="cv_t")
        nc.sync.dma_start(out=cv_t, in_=cv)
        bout_t = const.tile([1, DQ], bf16, name="bout_t")
        nc.sync.dma_start(out=bout_t, in_=bout)

        # ---- persistent SBUF tensors ----
        xgt_pool = ctx.enter_context(tc.tile_pool(name="xgt", bufs=KC, side="left"))
        xgt = [xgt_pool.tile([P, M], bf16, name=f"xgt{j}", tag="xgt") for j in range(KC)]
        xpt_pool = ctx.enter_context(tc.tile_pool(name="xpt", bufs=KQ, side="left"))
        xpt = [xpt_pool.tile([P, NPC], bf16, name=f"xpt{j}", tag="xpt") for j in range(KQ)]
        qt_pool = ctx.enter_context(tc.tile_pool(name="qt", bufs=KI, side="left"))
        qt = [qt_pool.tile([P, NPC], bf16, name=f"qt{i}", tag="qt") for i in range(KI)]
        kt_pool = ctx.enter_context(tc.tile_pool(name="kt", bufs=KI, side="left"))
        ktl = [kt_pool.tile([P, M], bf16, name=f"kt{i}", tag="kt") for i in range(KI)]
        v_pool = ctx.enter_context(tc.tile_pool(name="vsb", bufs=MT, side="left"))
        vt = [v_pool.tile([P, H, DH + 1], bf16, name=f"v{i}", tag="v") for i in range(MT)]
        att_pool = ctx.enter_context(tc.tile_pool(name="att", bufs=KI, side="left"))
        att = [att_pool.tile([P, NPC], bf16, name=f"att{i}", tag="att") for i in range(KI)]
        oxb_pool = ctx.enter_context(tc.tile_pool(name="oxb", bufs=16, side="left"))
        oxb = [oxb_pool.tile([P, 512], bf16, name=f"oxb{i}", tag="oxb") for i in range(16)]

        wv_pool = ctx.enter_context(tc.tile_pool(name="wvp", bufs=KC, side="left"))
        wv_t = [wv_pool.tile([P, INNER], bf16, name=f"wv{i}", tag="wv") for i in range(KC)]
        wft_pool = ctx.enter_context(tc.tile_pool(name="wftp", bufs=KQ, side="left"))
        wft_t = [wft_pool.tile([P, DQ], bf16, name=f"wft{i}", tag="wft") for i in range(KQ)]

        stat_pool = ctx.enter_context(tc.tile_pool(name="stat", bufs=1, side="left"))
        gcols = stat_pool.tile([P, MT, 3], f32, name="gcols")   # (-mu, std, rstd) per mt
        pcols = stat_pool.tile([P, NT, 3], f32, name="pcols")
        corr_g = stat_pool.tile([2, M], bf16, name="corr_g")    # rows: -mu, std over m
        corr_p = stat_pool.tile([2, NPC], bf16, name="corr_p")  # rows: -mu, std over n
        rstd_g_bc = stat_pool.tile([P, M], f32, name="rstd_g_bc")
        rstd_p_bc = stat_pool.tile([P, NPC], f32, name="rstd_p_bc")

        # ---- transient pools ----
        stage = ctx.enter_context(tc.tile_pool(name="stage", bufs=2, side="right"))
        wkq_pool = ctx.enter_context(tc.tile_pool(name="wkq", bufs=2, side="right"))
        ex_pool = ctx.enter_context(tc.tile_pool(name="ex", bufs=3, side="right"))
        rc_pool = ctx.enter_context(tc.tile_pool(name="rc", bufs=1, side="right"))
        bc_pool = ctx.enter_context(tc.tile_pool(name="bc", bufs=2, side="right"))
        out_pool = ctx.enter_context(tc.tile_pool(name="outp", bufs=2, side="right"))
        wof_pool = ctx.enter_context(tc.tile_pool(name="wofp", bufs=KI, side="right"))

        # ============ transposed raw inputs (start immediately) ============
        for j in range(KC):
            nc.sync.dma_start_transpose(xgt[j], xg[:, j * P:(j + 1) * P])
        for j in range(KQ):
            nc.sync.dma_start_transpose(xpt[j], xp[:, j * P:(j + 1) * P])

        # ============ weights ============
        for kt in range(KC):
            nc.sync.dma_start(out=wv_t[kt], in_=wv[kt * P:(kt + 1) * P, :])
        for kt in range(KQ):
            nc.sync.dma_start(out=wft_t[kt], in_=wft[kt * P:(kt + 1) * P, :])

        # ============ LN stats ============
        def stats_tile(x_dram, i, d, cols):
            """cols[:, i, :] = (-mu, std, rstd) for rows [i*128, (i+1)*128)."""
            x_t = stage.tile([P, d], bf16, tag="stx")
            nc.sync.dma_start(out=x_t, in_=x_dram[i * P:(i + 1) * P, :])
            fmax = min(nc.vector.BN_STATS_FMAX, d)
            while d % fmax:
                fmax //= 2
            nsub = d // fmax
            stats = small.tile([P, nsub, nc.vector.BN_STATS_DIM], f32, tag="stats")
            xv = x_t.rearrange("p (s f) -> p s f", s=nsub)
            for s in range(nsub):
                nc.vector.bn_stats(out=stats[:, s, :], in_=xv[:, s, :])
            mv = small.tile([P, nc.vector.BN_AGGR_DIM], f32, tag="mv")
            nc.vector.bn_aggr(out=mv, in_=stats)
            nc.vector.tensor_scalar_mul(cols[:, i, 0:1], mv[:, 0:1], -1.0)
            nc.scalar.activation(out=cols[:, i, 1:2], in_=mv[:, 1:2], func=AF.Sqrt, bias=eps_t)
            nc.vector.reciprocal(out=cols[:, i, 2:3], in_=cols[:, i, 1:2])

        def stats_flush(cols, ntile, stat_d):
            for i in range(ntile):
                # DRAM layout per tile: [3, 128] (rows -mu/std/rstd).
                dst = bass.AP(tensor=stat_d.tensor, offset=stat_d.offset + i * 3 * P,
                              ap=[[1, P], [P, 3]])
                nc.sync.dma_start(out=dst, in_=cols[:, i, :])

        for i in range(MT):
            stats_tile(xg, i, DC, gcols)
        stats_flush(gcols, MT, statg_d)
        for i in range(NT):
            stats_tile(xp, i, DQ, pcols)
        stats_flush(pcols, NT, statp_d)

        # Reload stats as rows: corr (bf16 [2, n]) + rstd broadcast ([128, n]).
        def stats_rows(stat_d, ntile, corr_bf, rstd_bc):
            corr_f = small.tile([2, ntile * P], f32, tag="corrf", bufs=1)
            nc.sync.dma_start(
                out=corr_f,
                in_=bass.AP(tensor=stat_d.tensor, offset=stat_d.offset,
                            ap=[[P, 2], [3 * P, ntile], [1, P]]),
            )
            nc.vector.tensor_copy(corr_bf, corr_f)
            nc.sync.dma_start(
                out=rstd_bc,
                in_=bass.AP(tensor=stat_d.tensor, offset=stat_d.offset + 2 * P,
                            ap=[[0, P], [3 * P, ntile], [1, P]]),
            )

        stats_rows(statg_d, MT, corr_g, rstd_g_bc)
        stats_rows(statp_d, NT, corr_p, rstd_p_bc)

        # ============ per-pair projection chains ============
        def emit_bk(p):
            wkc = wkq_pool.tile([P, KC, P], bf16, tag="wk")
            nc.sync.dma_start(
                out=wkc,
                in_=wk[:, p * P:(p + 1) * P].rearrange("(t p) c -> p t c", p=P),
            )
            for mch in range(M // 512):
                ps = ch_pool.tile([P, 512], f32, tag="ch")
                for kt in range(KC):
                    nc.tensor.matmul(
                        ps, wkc[:, kt, :], xgt[kt][:, mch * 512:(mch + 1) * 512],
                        start=(kt == 0), stop=False,
                    )
                nc.tensor.matmul(
                    ps, ck_t[:, p * P:(p + 1) * P],
                    corr_g[:, mch * 512:(mch + 1) * 512],
                    start=False, stop=True,
                )
                nc.vector.tensor_mul(
                    ktl[p][:, mch * 512:(mch + 1) * 512], ps,
                    rstd_g_bc[:, mch * 512:(mch + 1) * 512],
                )

        def emit_bq(p):
            wqc = wkq_pool.tile([P, KQ, P], bf16, tag="wq")
            nc.sync.dma_start(
                out=wqc,
                in_=wq[:, p * P:(p + 1) * P].rearrange("(t p) c -> p t c", p=P),
            )
            for nch in range(NPC // 512):
                ps = ch_pool.tile([P, 512], f32, tag="ch")
                for kt in range(KQ):
                    nc.tensor.matmul(
                        ps, wqc[:, kt, :], xpt[kt][:, nch * 512:(nch + 1) * 512],
                        start=(kt == 0), stop=False,
                    )
                nc.tensor.matmul(
                    ps, cq_t[:, p * P:(p + 1) * P],
                    corr_p[:, nch * 512:(nch + 1) * 512],
                    start=False, stop=True,
                )
                nc.vector.tensor_mul(
                    qt[p][:, nch * 512:(nch + 1) * 512], ps,
                    rstd_p_bc[:, nch * 512:(nch + 1) * 512],
                )

        def emit_bv(mt):
            nc.vector.memset(vt[mt][:, :, DH:DH + 1], 1.0)
            for ich in range(2):
                ps = ch_pool.tile([P, 512], f32, tag="ch")
                for kt in range(KC):
                    nc.tensor.matmul(
                        ps, xgt[kt][:, mt * P:(mt + 1) * P],
                        wv_t[kt][:, ich * 512:(ich + 1) * 512],
                        start=(kt == 0), stop=False,
                    )
                nc.tensor.matmul(
                    ps, corr_g[:, mt * P:(mt + 1) * P],
                    cv_t[:, ich * 512:(ich + 1) * 512],
                    start=False, stop=True,
                )
                nc.scalar.mul(
                    out=vt[mt][:, ich * 8:(ich + 1) * 8, 0:DH],
                    in_=ps.rearrange("p (h d) -> p h d", h=8),
                    mul=gcols[:, mt, 2:3],
                )

        def emit_dx(ch, nt):
            ps = ch_pool.tile([P, 512], f32, tag="ch")
            for kt in range(KQ):
                nc.tensor.matmul(
                    ps, xpt[kt][:, nt * P:(nt + 1) * P],
                    wft_t[kt][:, ch * 512:(ch + 1) * 512],
                    start=(kt == 0), stop=False,
                )
            nc.tensor.matmul(
                ps, ones_row, bout_t[:, ch * 512:(ch + 1) * 512],
                start=False, stop=True,
            )
            nc.vector.tensor_copy(oxb[ch * 8 + nt], ps)

        # ============ attention ============
        def emit_attn(p, nch):
            idx = p * 2 + nch
            pa = pa_pool.tile([P, 1024], f32, tag="pa")
            for mt in range(MT):
                sg = sg_pool.tile([P, 1024], f32, tag="sg")
                nc.tensor.matmul(
                    sg[:, 0:512],
                    ktl[p][0:DH, mt * P:(mt + 1) * P],
                    qt[p][0:DH, nch * 512:(nch + 1) * 512],
                )
                nc.tensor.matmul(
                    sg[:, 512:1024],
                    ktl[p][DH:P, mt * P:(mt + 1) * P],
                    qt[p][DH:P, nch * 512:(nch + 1) * 512],
                )
                ex = ex_pool.tile([P, 1024], bf16, tag="ex")
                nc.scalar.activation(out=ex, in_=sg, func=AF.Exp)
                nc.tensor.matmul(
                    pa[0:DH + 1, 0:512], vt[mt][:, 2 * p, :], ex[:, 0:512],
                    start=(mt == 0), stop=(mt == MT - 1),
                )
                nc.tensor.matmul(
                    pa[0:DH + 1, 512:1024], vt[mt][:, 2 * p + 1, :], ex[:, 512:1024],
                    start=(mt == 0), stop=(mt == MT - 1),
                )
            # Store UNNORMALIZED att (bf16) + denominators; softmax division
            # is deferred to one batched reciprocal at the end.
            rowtmp = rc_pool.tile([1, 1024], f32, tag="rc")
            nc.vector.tensor_copy(rowtmp, pa[DH:DH + 1, :])
            nc.sync.dma_start(out=rb_d[idx:idx + 1, :], in_=rowtmp)
            nc.vector.tensor_copy(
                att[p][0:DH, nch * 512:(nch + 1) * 512], pa[0:DH, 0:512]
            )
            nc.vector.tensor_copy(
                att[p][DH:P, nch * 512:(nch + 1) * 512], pa[0:DH, 512:1024]
            )

        # ============ program ============
        emit_bk(0)
        emit_bq(0)
        for mt in range(MT):
            emit_bv(mt)
        for p in range(NPAIR):
            if p + 1 < NPAIR:
                emit_bk(p + 1)
                emit_bq(p + 1)
            emit_attn(p, 0)
            emit_attn(p, 1)
            emit_dx(0, p)
            emit_dx(1, p)

        # ============ softmax normalization (batched) ============
        dn = small.tile([H, NPC], f32, tag="dn", bufs=1)
        nc.sync.dma_start(out=dn, in_=rb_d)
        nc.vector.reciprocal(out=dn, in_=dn)
        nc.sync.dma_start(out=rb2_d, in_=dn)
        for p in range(NPAIR):
            for nch in range(2):
                idx = p * 2 + nch
                bc = bc_pool.tile([P, 512], f32, tag="bc")
                nc.sync.dma_start(
                    out=bc,
                    in_=bass.AP(tensor=rb2_d.tensor,
                                offset=rb2_d.offset + idx * 1024,
                                ap=[[512, 2], [0, DH], [1, 512]]),
                )
                nc.vector.tensor_mul(
                    att[p][:, nch * 512:(nch + 1) * 512],
                    att[p][:, nch * 512:(nch + 1) * 512],
                    bc,
                )

        # ============ output: att @ WoF + residual part ============
        for ch in range(2):
            wofc = []
            for it in range(KI):
                w_t = wof_pool.tile([P, 512], bf16, tag="wof")
                nc.sync.dma_start(
                    out=w_t, in_=wof[it * P:(it + 1) * P, ch * 512:(ch + 1) * 512]
                )
                wofc.append(w_t)
            for nt in range(NT):
                ps = ch_pool.tile([P, 512], f32, tag="ch")
                for it in range(KI):
                    nc.tensor.matmul(
                        ps, att[it][:, nt * P:(nt + 1) * P],
                        wofc[it],
                        start=(it == 0), stop=(it == KI - 1),
                    )
                o_t = out_pool.tile([P, 512], f32, tag="o")
                nc.vector.tensor_add(o_t, ps, oxb[ch * 8 + nt])
                nc.sync.dma_start(
                    out=out[nt * P:(nt + 1) * P, ch * 512:(ch + 1) * 512],
                    in_=o_t,
                )

    nc.compile()
    return nc


def get_nc():
    if "nc" not in _CACHE:
        _CACHE["nc"] = _build_nc()
    return _CACHE["nc"]


def make_in_maps(inputs):
    """Host-side folding + sharding. Returns one input dict per core."""
    bf = ml_dtypes.bfloat16
    pf_ = np.asarray(inputs["person_features"], np.float32)
    gf_ = np.asarray(inputs["garment_features"], np.float32)
    Wq = np.asarray(inputs["Wq"], np.float32)
    Wk = np.asarray(inputs["Wk"], np.float32)
    Wv = np.asarray(inputs["Wv"], np.float32)
    Wo = np.asarray(inputs["Wo"], np.float32)
    bo = np.asarray(inputs["bo"], np.float32)
    Wf = np.asarray(inputs["Wf"], np.float32)
    bff = np.asarray(inputs["bf"], np.float32)
    gq = np.asarray(inputs["gq"], np.float32)
    betaq = np.asarray(inputs["betaq"], np.float32)
    gk = np.asarray(inputs["gk"], np.float32)
    betak = np.asarray(inputs["betak"], np.float32)

    wq_f = ((gq[:, None] * Wq) * np.float32(SCALE)).astype(bf)
    bq_f = (betaq @ Wq) * np.float32(SCALE)
    wk_f = (gk[:, None] * Wk).astype(bf)
    bk_f = betak @ Wk
    wv_f = (gk[:, None] * Wv).astype(bf)
    bv_f = betak @ Wv
    wf_top = np.ascontiguousarray(Wf[:DQ])
    wf_bot = Wf[DQ:]
    wof = (Wo.astype(np.float64) @ wf_bot.astype(np.float64)).astype(np.float32)
    bout = (bo @ wf_bot + bff).astype(np.float32)

    # Correction rows: [colsum(W'); bias] per projection (colsums of the
    # bf16-rounded weights actually used on-device).
    def corr(w_b, bias):
        cs = w_b.astype(np.float64).sum(axis=0).astype(np.float32)
        return np.stack([cs, bias]).astype(bf)

    shared = {
        "wq": wq_f,
        "wk": wk_f,
        "wv": wv_f,
        "wof": wof.astype(bf),
        "wft": wf_top.astype(bf),
        "cq": corr(wq_f, bq_f),
        "ck": corr(wk_f, bk_f),
        "cv": corr(wv_f, bv_f),
        "bout": bout[None, :].astype(bf),
    }
    in_maps = []
    for core in range(NCORES):
        b, half = divmod(core, 2)
        m = dict(shared)
        m["xp"] = np.ascontiguousarray(pf_[b, half * NPC:(half + 1) * NPC]).astype(bf)
        m["xg"] = np.ascontiguousarray(gf_[b]).astype(bf)
        in_maps.append(m)
    return in_maps


def assemble(results):
    out = np.empty((B, N, DQ), np.float32)
    for core in range(NCORES):
        b, half = divmod(core, 2)
        out[b, half * NPC:(half + 1) * NPC] = results[core]["out"]
    return out


def kernel(**inputs):
    from concourse.bass_utils import run_bass_kernel_spmd

    nc = get_nc()
    in_maps = make_in_maps(inputs)
    res = run_bass_kernel_spmd(nc, in_maps, list(range(NCORES)))
    return assemble(res.results)


# revision 20
# speedup vs baseline: 1.0314x; 1.0314x over previous
"""Trainium2 Bass kernel: GarmentPersonCrossAttention (B=4, N=2048, M=1024,
DQ=1024, DC=768, H=16, DH=64), distributed over 8 NeuronCores.

Sharding: core i handles batch i//2 and person-row half i%2 (1024 rows).
Everything is local per core (garment-side work is recomputed by both
cores of a batch pair) -- no collectives.

Host-side algebraic folds (exact linear algebra, numpy):
  - LN affine (gamma) folded into Wq/Wk/Wv; softmax scale into Wq.
  - concat([residual, att]) @ Wf + bf
        = residual @ Wf[:DQ] + att @ (Wo @ Wf[DQ:]) + (bo @ Wf[DQ:] + bf)
  - LayerNorm itself is folded into the projections:
        q[n,i] = rstd[n] * ( (x @ Wq')[n,i] - mu[n]*colsum_q[i] + std[n]*bq[i] )
    so projections run on the RAW transposed inputs plus one K=2
    "correction" matmul (rows: -mu, std), and the per-row rstd is applied
    during PSUM evacuation (DVE multiply against a broadcast row for the
    feature-major q/k, ACT per-partition scale for the row-major v).

Device pipeline per core (bf16 matmuls, fp32 PSUM accumulation):
  - DMA-transpose raw xp/xg into feature-major xpT/xgT (no LN round trip).
  - LN stats (bn_stats) on row-major tiles -> -mu/std/rstd columns;
    DMA-bounced into rows for the correction matmuls + rstd broadcasts.
  - Per head-pair p: kT/qT projection chains (6-8 MMs + corr MM, DVE
    rstd evac), then attention: per mt, two row-tiled concurrent score
    MMs (K=64 at array rows 0-63 / 64-127) into a 2-bank PSUM group,
    ONE 1024-wide exp (ACT), two attV MMs accumulating into a 2-bank pa
    (ones column yields denominators at partition 64).  Softmax
    normalization via reciprocal_approx_fast on [1,1024] + DMA broadcast
    bounce.  Residual-path output chains (xT @ Wf_top) interleave as PE
    filler; att @ (Wo Wf_bot) chains run at the end.
"""

import os
import sys

import numpy as np

for _p in ("/opt/trn_rl_repo",):
    if _p not in sys.path and os.path.isdir(_p):
        sys.path.append(_p)

import ml_dtypes

# Problem constants (hardcoded per contest rules).
B, N, M = 4, 2048, 1024
DQ, DC = 1024, 768
H, DH = 16, 64
INNER = H * DH
SCALE = DH ** -0.5
EPS = 1e-5
NCORES = 8
NPC = N // 2          # person rows per core
P = 128               # partitions
NT = NPC // P         # 8 person row tiles per core
MT = M // P           # 8 garment row tiles
KQ = DQ // P          # 8 contraction tiles for person features
KC = DC // P          # 6 contraction tiles for garment features
KI = INNER // P       # 8 inner tiles
NPAIR = H // 2        # 8 head pairs (pair p = heads 2p, 2p+1 = inner tile p)

_CACHE = {}


def _build_nc():
    import concourse.bass as bass
    import concourse.tile as tile
    from concourse import bacc, mybir
    from contextlib import ExitStack

    f32 = mybir.dt.float32
    bf16 = mybir.dt.bfloat16
    AF = mybir.ActivationFunctionType

    nc = bacc.Bacc("TRN2", target_bir_lowering=False, debug=False)

    # ---- DRAM parameters (per-core shards; weights replicated) ----
    xp = nc.dram_tensor("xp", [NPC, DQ], bf16, kind="ExternalInput").ap()
    xg = nc.dram_tensor("xg", [M, DC], bf16, kind="ExternalInput").ap()
    wq = nc.dram_tensor("wq", [DQ, INNER], bf16, kind="ExternalInput").ap()
    wk = nc.dram_tensor("wk", [DC, INNER], bf16, kind="ExternalInput").ap()
    wv = nc.dram_tensor("wv", [DC, INNER], bf16, kind="ExternalInput").ap()
    wof = nc.dram_tensor("wof", [INNER, DQ], bf16, kind="ExternalInput").ap()
    wft = nc.dram_tensor("wft", [DQ, DQ], bf16, kind="ExternalInput").ap()
    cq = nc.dram_tensor("cq", [2, INNER], bf16, kind="ExternalInput").ap()
    ck = nc.dram_tensor("ck", [2, INNER], bf16, kind="ExternalInput").ap()
    cv = nc.dram_tensor("cv", [2, INNER], bf16, kind="ExternalInput").ap()
    bout = nc.dram_tensor("bout", [1, DQ], bf16, kind="ExternalInput").ap()
    selc = nc.dram_tensor("selc", [2, P], f32, kind="ExternalInput").ap()
    out = nc.dram_tensor("out", [NPC, DQ], f32, kind="ExternalOutput").ap()

    # Internal DRAM scratch.
    # Per row-tile stats [mt, (negmu, std, rstd), 128] for each side.
    statp_d = nc.dram_tensor("statp_scratch", [NT * 3 * P], f32).ap()
    statg_d = nc.dram_tensor("statg_scratch", [MT * 3 * P], f32).ap()
    rb_d = nc.dram_tensor("denom_scratch", [H, NPC], f32).ap()
    rb2_d = nc.dram_tensor("recip_scratch", [H, NPC], f32).ap()

    with tile.TileContext(nc) as tc, ExitStack() as ctx:
        # PSUM: pa (2 banks) + score groups (2x2 banks) + chains (2x1).
        pa_pool = ctx.enter_context(tc.tile_pool(name="pa", bufs=1, space="PSUM"))
        sg_pool = ctx.enter_context(tc.tile_pool(name="sg", bufs=2, space="PSUM"))
        ch_pool = ctx.enter_context(tc.tile_pool(name="chain", bufs=2, space="PSUM"))

        const = ctx.enter_context(tc.tile_pool(name="const", bufs=1, side="left"))
        small = ctx.enter_context(tc.tile_pool(name="small", bufs=4, side="left"))

        # ---- constants ----
        eps_t = const.tile([P, 1], f32, name="eps_t")
        nc.vector.memset(eps_t, EPS)
        ones_row = const.tile([1, P], bf16, name="ones_row")
        nc.vector.memset(ones_row, 1.0)
        ones_f = const.tile([1, P], f32, name="ones_f")
        nc.vector.memset(ones_f, 1.0)
        # Head-half selector for the softmax-normalization broadcast matmul.
        sel = const.tile([2, P], f32, name="sel")
        nc.sync.dma_start(out=sel, in_=selc)
        cq_t = const.tile([2, INNER], bf16, name="cq_t")
        nc.sync.dma_start(out=cq_t, in_=cq)
        ck_t = const.tile([2, INNER], bf16, name="ck_t")
        nc.sync.dma_start(out=ck_t, in_=ck)
        cv_t = const.tile([2, INNER], bf16, name="cv_t")
        nc.sync.dma_start(out=cv_t, in_=cv)
        bout_t = const.tile([1, DQ], bf16, name="bout_t")
        nc.sync.dma_start(out=bout_t, in_=bout)

        # ---- persistent SBUF tensors ----
        xgt_pool = ctx.enter_context(tc.tile_pool(name="xgt", bufs=KC, side="left"))
        xgt = [xgt_pool.tile([P, M], bf16, name=f"xgt{j}", tag="xgt") for j in range(KC)]
        xpt_pool = ctx.enter_context(tc.tile_pool(name="xpt", bufs=KQ, side="left"))
        xpt = [xpt_pool.tile([P, NPC], bf16, name=f"xpt{j}", tag="xpt") for j in range(KQ)]
        qt_pool = ctx.enter_context(tc.tile_pool(name="qt", bufs=KI, side="left"))
        qt = [qt_pool.tile([P, NPC], bf16, name=f"qt{i}", tag="qt") for i in range(KI)]
        kt_pool = ctx.enter_context(tc.tile_pool(name="kt", bufs=KI, side="left"))
        ktl = [kt_pool.tile([P, M], bf16, name=f"kt{i}", tag="kt") for i in range(KI)]
        v_pool = ctx.enter_context(tc.tile_pool(name="vsb", bufs=MT, side="left"))
        vt = [v_pool.tile([P, H, DH + 1], bf16, name=f"v{i}", tag="v") for i in range(MT)]
        att_pool = ctx.enter_context(tc.tile_pool(name="att", bufs=KI, side="left"))
        att = [att_pool.tile([P, NPC], bf16, name=f"att{i}", tag="att") for i in range(KI)]
        oxb_pool = ctx.enter_context(tc.tile_pool(name="oxb", bufs=16, side="left"))
        oxb = [oxb_pool.tile([P, 512], bf16, name=f"oxb{i}", tag="oxb") for i in range(16)]

        wv_pool = ctx.enter_context(tc.tile_pool(name="wvp", bufs=KC, side="left"))
        wv_t = [wv_pool.tile([P, INNER], bf16, name=f"wv{i}", tag="wv") for i in range(KC)]
        wft_pool = ctx.enter_context(tc.tile_pool(name="wftp", bufs=KQ, side="left"))
        wft_t = [wft_pool.tile([P, DQ], bf16, name=f"wft{i}", tag="wft") for i in range(KQ)]

        stat_pool = ctx.enter_context(tc.tile_pool(name="stat", bufs=1, side="left"))
        gcols = stat_pool.tile([P, MT, 3], f32, name="gcols")   # (-mu, std, rstd) per mt
        pcols = stat_pool.tile([P, NT, 3], f32, name="pcols")
        corr_g = stat_pool.tile([2, M], bf16, name="corr_g")    # rows: -mu, std over m
        corr_p = stat_pool.tile([2, NPC], bf16, name="corr_p")  # rows: -mu, std over n
        rstd_g_bc = stat_pool.tile([P, M], f32, name="rstd_g_bc")
        rstd_p_bc = stat_pool.tile([P, NPC], f32, name="rstd_p_bc")

        # ---- transient pools ----
        stage = ctx.enter_context(tc.tile_pool(name="stage", bufs=2, side="right"))
        wkq_pool = ctx.enter_context(tc.tile_pool(name="wkq", bufs=2, side="right"))
        ex_pool = ctx.enter_context(tc.tile_pool(name="ex", bufs=2, side="right"))
        rc_pool = ctx.enter_context(tc.tile_pool(name="rc", bufs=1, side="right"))
        out_pool = ctx.enter_context(tc.tile_pool(name="outp", bufs=2, side="right"))
        wof_pool = ctx.enter_context(tc.tile_pool(name="wofp", bufs=KI, side="right"))

        # ============ LN stats ============
        def stats_tile(x_dram, i, d, cols):
            """cols[:, i, :] = (-mu, std, rstd) for rows [i*128, (i+1)*128)."""
            x_t = stage.tile([P, d], bf16, tag="stx")
            nc.sync.dma_start(out=x_t, in_=x_dram[i * P:(i + 1) * P, :])
            fmax = min(nc.vector.BN_STATS_FMAX, d)
            while d % fmax:
                fmax //= 2
            nsub = d // fmax
            stats = small.tile([P, nsub, nc.vector.BN_STATS_DIM], f32, tag="stats")
            xv = x_t.rearrange("p (s f) -> p s f", s=nsub)
            for s in range(nsub):
                nc.vector.bn_stats(out=stats[:, s, :], in_=xv[:, s, :])
            mv = small.tile([P, nc.vector.BN_AGGR_DIM], f32, tag="mv")
            nc.vector.bn_aggr(out=mv, in_=stats)
            nc.vector.tensor_scalar_mul(cols[:, i, 0:1], mv[:, 0:1], -1.0)
            nc.scalar.activation(out=cols[:, i, 1:2], in_=mv[:, 1:2], func=AF.Sqrt, bias=eps_t)
            nc.vector.reciprocal(out=cols[:, i, 2:3], in_=cols[:, i, 1:2])

        def stats_flush(cols, ntile, stat_d):
            for i in range(ntile):
                # DRAM layout per tile: [3, 128] (rows -mu/std/rstd).
                dst = bass.AP(tensor=stat_d.tensor, offset=stat_d.offset + i * 3 * P,
                              ap=[[1, P], [P, 3]])
                nc.sync.dma_start(out=dst, in_=cols[:, i, :])

        # Reload stats as rows: corr (bf16 [2, n]); rstd broadcast built with
        # a K=1 ones-matmul into PSUM (partition-broadcast DMAs are slow).
        def stats_rows(stat_d, ntile, corr_bf, rstd_bc):
            corr_f = small.tile([2, ntile * P], f32, tag="scr4k", bufs=1)
            nc.sync.dma_start(
                out=corr_f,
                in_=bass.AP(tensor=stat_d.tensor, offset=stat_d.offset,
                            ap=[[P, 2], [3 * P, ntile], [1, P]]),
            )
            nc.vector.tensor_copy(corr_bf, corr_f)
            rrow = small.tile([1, ntile * P], f32, tag="scr4k", bufs=1)
            nc.sync.dma_start(
                out=rrow,
                in_=bass.AP(tensor=stat_d.tensor, offset=stat_d.offset + 2 * P,
                            ap=[[3 * P, ntile], [1, P]]),
            )
            for half in range(ntile * P // 512):
                bb = ch_pool.tile([P, 512], f32, tag="ch")
                nc.tensor.matmul(
                    bb, ones_f, rrow[0:1, half * 512:(half + 1) * 512],
                    start=True, stop=True,
                )
                nc.vector.tensor_copy(rstd_bc[:, half * 512:(half + 1) * 512], bb)

        # Prologue order: garment-side stream gates the first attention, so
        # it goes first.  The stats pipeline rides the sync queue; the
        # scalar (ACT) queue -- idle until the first exp -- carries the
        # transposes and weights in parallel.
        for i in range(MT):
            stats_tile(xg, i, DC, gcols)
        for j in range(KC):
            nc.scalar.dma_start_transpose(xgt[j], xg[:, j * P:(j + 1) * P])
        for kt in range(KC):
            nc.scalar.dma_start(out=wv_t[kt], in_=wv[kt * P:(kt + 1) * P, :])
        stats_flush(gcols, MT, statg_d)
        stats_rows(statg_d, MT, corr_g, rstd_g_bc)
        for i in range(NT):
            stats_tile(xp, i, DQ, pcols)
        for j in range(KQ):
            nc.scalar.dma_start_transpose(xpt[j], xp[:, j * P:(j + 1) * P])
        stats_flush(pcols, NT, statp_d)
        stats_rows(statp_d, NT, corr_p, rstd_p_bc)
        for kt in range(KQ):
            nc.scalar.dma_start(out=wft_t[kt], in_=wft[kt * P:(kt + 1) * P, :])

        # ============ per-pair projection chains ============
        def emit_bk(p):
            wkc = wkq_pool.tile([P, KC, P], bf16, tag="wk")
            nc.sync.dma_start(
                out=wkc,
                in_=wk[:, p * P:(p + 1) * P].rearrange("(t p) c -> p t c", p=P),
            )
            for mch in range(M // 512):
                ps = ch_pool.tile([P, 512], f32, tag="ch")
                for kt in range(KC):
                    nc.tensor.matmul(
                        ps, wkc[:, kt, :], xgt[kt][:, mch * 512:(mch + 1) * 512],
                        start=(kt == 0), stop=False,
                    )
                nc.tensor.matmul(
                    ps, ck_t[:, p * P:(p + 1) * P],
                    corr_g[:, mch * 512:(mch + 1) * 512],
                    start=False, stop=True,
                )
                nc.vector.tensor_mul(
                    ktl[p][:, mch * 512:(mch + 1) * 512], ps,
                    rstd_g_bc[:, mch * 512:(mch + 1) * 512],
                )

        def emit_bq(p):
            wqc = wkq_pool.tile([P, KQ, P], bf16, tag="wq")
            nc.sync.dma_start(
                out=wqc,
                in_=wq[:, p * P:(p + 1) * P].rearrange("(t p) c -> p t c", p=P),
            )
            for nch in range(NPC // 512):
                ps = ch_pool.tile([P, 512], f32, tag="ch")
                for kt in range(KQ):
                    nc.tensor.matmul(
                        ps, wqc[:, kt, :], xpt[kt][:, nch * 512:(nch + 1) * 512],
                        start=(kt == 0), stop=False,
                    )
                nc.tensor.matmul(
                    ps, cq_t[:, p * P:(p + 1) * P],
                    corr_p[:, nch * 512:(nch + 1) * 512],
                    start=False, stop=True,
                )
                nc.vector.tensor_mul(
                    qt[p][:, nch * 512:(nch + 1) * 512], ps,
                    rstd_p_bc[:, nch * 512:(nch + 1) * 512],
                )

        def emit_bv(mt):
            nc.vector.memset(vt[mt][:, :, DH:DH + 1], 1.0)
            for ich in range(2):
                ps = ch_pool.tile([P, 512], f32, tag="ch")
                for kt in range(KC):
                    nc.tensor.matmul(
                        ps, xgt[kt][:, mt * P:(mt + 1) * P],
                        wv_t[kt][:, ich * 512:(ich + 1) * 512],
                        start=(kt == 0), stop=False,
                    )
                nc.tensor.matmul(
                    ps, corr_g[:, mt * P:(mt + 1) * P],
                    cv_t[:, ich * 512:(ich + 1) * 512],
                    start=False, stop=True,
                )
                nc.vector.tensor_scalar_mul(
                    vt[mt][:, ich * 8:(ich + 1) * 8, 0:DH],
                    ps.rearrange("p (h d) -> p h d", h=8),
                    gcols[:, mt, 2:3],
                )

        def emit_dx(ch, nt):
            ps = ch_pool.tile([P, 512], f32, tag="ch")
            for kt in range(KQ):
                nc.tensor.matmul(
                    ps, xpt[kt][:, nt * P:(nt + 1) * P],
                    wft_t[kt][:, ch * 512:(ch + 1) * 512],
                    start=(kt == 0), stop=False,
                )
            nc.tensor.matmul(
                ps, ones_row, bout_t[:, ch * 512:(ch + 1) * 512],
                start=False, stop=True,
            )
            nc.vector.tensor_copy(oxb[ch * 8 + nt], ps)

        # ============ attention ============
        def emit_attn(p, nch):
            idx = p * 2 + nch
            pa = pa_pool.tile([P, 1024], f32, tag="pa")
            for mt in range(MT):
                sg = sg_pool.tile([P, 1024], f32, tag="sg")
                nc.tensor.matmul(
                    sg[:, 0:512],
                    ktl[p][0:DH, mt * P:(mt + 1) * P],
                    qt[p][0:DH, nch * 512:(nch + 1) * 512],
                )
                nc.tensor.matmul(
                    sg[:, 512:1024],
                    ktl[p][DH:P, mt * P:(mt + 1) * P],
                    qt[p][DH:P, nch * 512:(nch + 1) * 512],
                )
                ex = ex_pool.tile([P, 1024], bf16, tag="ex")
                nc.scalar.activation(out=ex, in_=sg, func=AF.Exp)
                nc.tensor.matmul(
                    pa[0:DH + 1, 0:512], vt[mt][:, 2 * p, :], ex[:, 0:512],
                    start=(mt == 0), stop=(mt == MT - 1),
                )
                nc.tensor.matmul(
                    pa[0:DH + 1, 512:1024], vt[mt][:, 2 * p + 1, :], ex[:, 512:1024],
                    start=(mt == 0), stop=(mt == MT - 1),
                )
            # Store UNNORMALIZED att (bf16) + denominators; softmax division
            # is deferred to one batched reciprocal at the end.
            rowtmp = rc_pool.tile([1, 1024], f32, tag="rc")
            nc.vector.tensor_copy(rowtmp, pa[DH:DH + 1, :])
            nc.sync.dma_start(out=rb_d[idx:idx + 1, :], in_=rowtmp)
            nc.vector.tensor_copy(
                att[p][0:DH, nch * 512:(nch + 1) * 512], pa[0:DH, 0:512]
            )
            nc.vector.tensor_copy(
                att[p][DH:P, nch * 512:(nch + 1) * 512], pa[0:DH, 512:1024]
            )

        # ============ program ============
        emit_bk(0)
        emit_bq(0)
        for mt in range(MT):
            emit_bv(mt)
        wofc = []
        for p in range(NPAIR):
            emit_attn(p, 0)
            emit_attn(p, 1)
            if p + 1 < NPAIR:
                emit_bk(p + 1)
                emit_bq(p + 1)
            if p >= 1:
                emit_dx(0, p - 1)
                emit_dx(1, p - 1)
            if p == 5:
                for it in range(KI):
                    w_t = wof_pool.tile([P, DQ], bf16, tag="wof")
                    nc.scalar.dma_start(out=w_t, in_=wof[it * P:(it + 1) * P, :])
                    wofc.append(w_t)
        emit_dx(0, NT - 1)
        emit_dx(1, NT - 1)

        # ============ softmax normalization + output ============
        # One batched reciprocal over all denominators; broadcast across
        # partitions via a K=2 selector matmul into PSUM (no DMA).
        dn = small.tile([2 * H, 512], f32, tag="scr4k", bufs=1)
        nc.sync.dma_start(
            out=dn,
            in_=bass.AP(tensor=rb_d.tensor, offset=rb_d.offset,
                        ap=[[512, 2 * H], [1, 512]]),
        )
        nc.vector.reciprocal(out=dn, in_=dn)
        nc.sync.dma_start(
            out=bass.AP(tensor=rb2_d.tensor, offset=rb2_d.offset,
                        ap=[[512, 2 * H], [1, 512]]),
            in_=dn,
        )
        for nch in range(2):
            for p in range(NPAIR):
                idx = p * 2 + nch
                dnp = rc_pool.tile([2, 512], f32, tag="dnp", bufs=2)
                nc.sync.dma_start(
                    out=dnp,
                    in_=bass.AP(tensor=rb2_d.tensor,
                                offset=rb2_d.offset + idx * 1024,
                                ap=[[512, 2], [1, 512]]),
                )
                bcp = pa_pool.tile([P, 1024], f32, tag="pa")
                nc.tensor.matmul(
                    bcp[:, 0:512], sel, dnp,
                    start=True, stop=True,
                )
                nc.vector.tensor_mul(
                    att[p][:, nch * 512:(nch + 1) * 512],
                    att[p][:, nch * 512:(nch + 1) * 512],
                    bcp[:, 0:512],
                )
            for nt in range(nch * 4, nch * 4 + 4):
                for ch in range(2):
                    ps = ch_pool.tile([P, 512], f32, tag="ch")
                    for it in range(KI):
                        nc.tensor.matmul(
                            ps, att[it][:, nt * P:(nt + 1) * P],
                            wofc[it][:, ch * 512:(ch + 1) * 512],
                            start=(it == 0), stop=(it == KI - 1),
                        )
                    o_t = out_pool.tile([P, 512], f32, tag="o")
                    nc.vector.tensor_add(o_t, ps, oxb[ch * 8 + nt])
                    nc.sync.dma_start(
                        out=out[nt * P:(nt + 1) * P, ch * 512:(ch + 1) * 512],
                        in_=o_t,
                    )

    nc.compile()
    return nc


def get_nc():
    if "nc" not in _CACHE:
        _CACHE["nc"] = _build_nc()
    return _CACHE["nc"]


def make_in_maps(inputs):
    """Host-side folding + sharding. Returns one input dict per core."""
    bf = ml_dtypes.bfloat16
    pf_ = np.asarray(inputs["person_features"], np.float32)
    gf_ = np.asarray(inputs["garment_features"], np.float32)
    Wq = np.asarray(inputs["Wq"], np.float32)
    Wk = np.asarray(inputs["Wk"], np.float32)
    Wv = np.asarray(inputs["Wv"], np.float32)
    Wo = np.asarray(inputs["Wo"], np.float32)
    bo = np.asarray(inputs["bo"], np.float32)
    Wf = np.asarray(inputs["Wf"], np.float32)
    bff = np.asarray(inputs["bf"], np.float32)
    gq = np.asarray(inputs["gq"], np.float32)
    betaq = np.asarray(inputs["betaq"], np.float32)
    gk = np.asarray(inputs["gk"], np.float32)
    betak = np.asarray(inputs["betak"], np.float32)

    wq_f = ((gq[:, None] * Wq) * np.float32(SCALE)).astype(bf)
    bq_f = (betaq @ Wq) * np.float32(SCALE)
    wk_f = (gk[:, None] * Wk).astype(bf)
    bk_f = betak @ Wk
    wv_f = (gk[:, None] * Wv).astype(bf)
    bv_f = betak @ Wv
    wf_top = np.ascontiguousarray(Wf[:DQ])
    wf_bot = Wf[DQ:]
    wof = (Wo.astype(np.float64) @ wf_bot.astype(np.float64)).astype(np.float32)
    bout = (bo @ wf_bot + bff).astype(np.float32)

    # Correction rows: [colsum(W'); bias] per projection (colsums of the
    # bf16-rounded weights actually used on-device).
    def corr(w_b, bias):
        cs = w_b.astype(np.float64).sum(axis=0).astype(np.float32)
        return np.stack([cs, bias]).astype(bf)

    shared = {
        "wq": wq_f,
        "wk": wk_f,
        "wv": wv_f,
        "wof": wof.astype(bf),
        "wft": wf_top.astype(bf),
        "cq": corr(wq_f, bq_f),
        "ck": corr(wk_f, bk_f),
        "cv": corr(wv_f, bv_f),
        "bout": bout[None, :].astype(bf),
        "selc": np.concatenate(
            [np.repeat([[1.0, 0.0]], DH, 0), np.repeat([[0.0, 1.0]], DH, 0)]
        ).T.astype(np.float32).copy(),
    }
    in_maps = []
    for core in range(NCORES):
        b, half = divmod(core, 2)
        m = dict(shared)
        m["xp"] = np.ascontiguousarray(pf_[b, half * NPC:(half + 1) * NPC]).astype(bf)
        m["xg"] = np.ascontiguousarray(gf_[b]).astype(bf)
        in_maps.append(m)
    return in_maps


def assemble(results):
    out = np.empty((B, N, DQ), np.float32)
    for core in range(NCORES):
        b, half = divmod(core, 2)
        out[b, half * NPC:(half + 1) * NPC] = results[core]["out"]
    return out


def kernel(**inputs):
    from concourse.bass_utils import run_bass_kernel_spmd

    nc = get_nc()
    in_maps = make_in_maps(inputs)
    res = run_bass_kernel_spmd(nc, in_maps, list(range(NCORES)))
    return assemble(res.results)


# revision 21
# speedup vs baseline: 1.1034x; 1.0699x over previous
"""Trainium2 Bass kernel: GarmentPersonCrossAttention (B=4, N=2048, M=1024,
DQ=1024, DC=768, H=16, DH=64), distributed over 8 NeuronCores.

Sharding: core i handles batch i//2 and person-row half i%2 (1024 rows).
Everything is local per core (garment-side work is recomputed by both
cores of a batch pair) -- no collectives.

Host-side algebraic folds (exact linear algebra, numpy):
  - LN affine (gamma) folded into Wq/Wk/Wv; softmax scale into Wq.
  - concat([residual, att]) @ Wf + bf
        = residual @ Wf[:DQ] + att @ (Wo @ Wf[DQ:]) + (bo @ Wf[DQ:] + bf)
  - LayerNorm itself is folded into the projections:
        q[n,i] = rstd[n] * ( (x @ Wq')[n,i] - mu[n]*colsum_q[i] + std[n]*bq[i] )
    so projections run on the RAW transposed inputs plus one K=2
    "correction" matmul (rows: -mu, std), and the per-row rstd is applied
    during PSUM evacuation (DVE multiply against a broadcast row for the
    feature-major q/k, ACT per-partition scale for the row-major v).

Device pipeline per core (bf16 matmuls, fp32 PSUM accumulation):
  - DMA-transpose raw xp/xg into feature-major xpT/xgT (no LN round trip).
  - LN stats (bn_stats) on row-major tiles -> -mu/std/rstd columns;
    DMA-bounced into rows for the correction matmuls + rstd broadcasts.
  - Per head-pair p: kT/qT projection chains (6-8 MMs + corr MM, DVE
    rstd evac), then attention: per mt, two row-tiled concurrent score
    MMs (K=64 at array rows 0-63 / 64-127) into a 2-bank PSUM group,
    ONE 1024-wide exp (ACT), two attV MMs accumulating into a 2-bank pa
    (ones column yields denominators at partition 64).  Softmax
    normalization via reciprocal_approx_fast on [1,1024] + DMA broadcast
    bounce.  Residual-path output chains (xT @ Wf_top) interleave as PE
    filler; att @ (Wo Wf_bot) chains run at the end.
"""

import os
import sys

import numpy as np

for _p in ("/opt/trn_rl_repo",):
    if _p not in sys.path and os.path.isdir(_p):
        sys.path.append(_p)

import ml_dtypes

# Problem constants (hardcoded per contest rules).
B, N, M = 4, 2048, 1024
DQ, DC = 1024, 768
H, DH = 16, 64
INNER = H * DH
SCALE = DH ** -0.5
EPS = 1e-5
NCORES = 8
NPC = N // 2          # person rows per core
P = 128               # partitions
NT = NPC // P         # 8 person row tiles per core
MT = M // P           # 8 garment row tiles
KQ = DQ // P          # 8 contraction tiles for person features
KC = DC // P          # 6 contraction tiles for garment features
KI = INNER // P       # 8 inner tiles
NPAIR = H // 2        # 8 head pairs (pair p = heads 2p, 2p+1 = inner tile p)

_CACHE = {}


def _build_nc():
    import concourse.bass as bass
    import concourse.tile as tile
    from concourse import bacc, mybir
    from contextlib import ExitStack

    f32 = mybir.dt.float32
    bf16 = mybir.dt.bfloat16
    AF = mybir.ActivationFunctionType

    nc = bacc.Bacc("TRN2", target_bir_lowering=False, debug=False)

    # ---- DRAM parameters (per-core shards; weights replicated) ----
    xp = nc.dram_tensor("xp", [NPC, DQ], bf16, kind="ExternalInput").ap()
    xg = nc.dram_tensor("xg", [M, DC], bf16, kind="ExternalInput").ap()
    wq = nc.dram_tensor("wq", [DQ, INNER], bf16, kind="ExternalInput").ap()
    wk = nc.dram_tensor("wk", [DC, INNER], bf16, kind="ExternalInput").ap()
    wv = nc.dram_tensor("wv", [DC, INNER], bf16, kind="ExternalInput").ap()
    wof = nc.dram_tensor("wof", [INNER, DQ], bf16, kind="ExternalInput").ap()
    wft = nc.dram_tensor("wft", [DQ, DQ], bf16, kind="ExternalInput").ap()
    cq = nc.dram_tensor("cq", [2, INNER], bf16, kind="ExternalInput").ap()
    ck = nc.dram_tensor("ck", [2, INNER], bf16, kind="ExternalInput").ap()
    cv = nc.dram_tensor("cv", [2, INNER], bf16, kind="ExternalInput").ap()
    bout = nc.dram_tensor("bout", [1, DQ], bf16, kind="ExternalInput").ap()
    selc = nc.dram_tensor("selc", [2, P], f32, kind="ExternalInput").ap()
    idn = nc.dram_tensor("idn", [P, P], bf16, kind="ExternalInput").ap()
    out = nc.dram_tensor("out", [NPC, DQ], f32, kind="ExternalOutput").ap()

    # Internal DRAM scratch.
    # Per row-tile stats [mt, (negmu, std, rstd), 128] for each side.
    statp_d = nc.dram_tensor("statp_scratch", [NT * 3 * P], f32).ap()
    statg_d = nc.dram_tensor("statg_scratch", [MT * 3 * P], f32).ap()
    rb_d = nc.dram_tensor("denom_scratch", [H, NPC], f32).ap()
    rb2_d = nc.dram_tensor("recip_scratch", [H, NPC], f32).ap()

    with tile.TileContext(nc) as tc, ExitStack() as ctx:
        # PSUM: pa (2 banks) + score groups (2x2 banks) + chains (2x1).
        pa_pool = ctx.enter_context(tc.tile_pool(name="pa", bufs=1, space="PSUM"))
        sg_pool = ctx.enter_context(tc.tile_pool(name="sg", bufs=2, space="PSUM"))
        ch_pool = ctx.enter_context(tc.tile_pool(name="chain", bufs=2, space="PSUM"))

        const = ctx.enter_context(tc.tile_pool(name="const", bufs=1, side="left"))
        small = ctx.enter_context(tc.tile_pool(name="small", bufs=4, side="left"))

        # ---- constants ----
        eps_t = const.tile([P, 1], f32, name="eps_t")
        nc.vector.memset(eps_t, EPS)
        ones_row = const.tile([1, P], bf16, name="ones_row")
        nc.vector.memset(ones_row, 1.0)
        ones_f = const.tile([1, P], f32, name="ones_f")
        nc.vector.memset(ones_f, 1.0)
        # Head-half selector for the softmax-normalization broadcast matmul.
        sel = const.tile([2, P], f32, name="sel")
        nc.sync.dma_start(out=sel, in_=selc)
        idn_t = const.tile([P, P], bf16, name="idn_t")
        nc.sync.dma_start(out=idn_t, in_=idn)
        cq_t = const.tile([2, INNER], bf16, name="cq_t")
        nc.sync.dma_start(out=cq_t, in_=cq)
        ck_t = const.tile([2, INNER], bf16, name="ck_t")
        nc.sync.dma_start(out=ck_t, in_=ck)
        cv_t = const.tile([2, INNER], bf16, name="cv_t")
        nc.sync.dma_start(out=cv_t, in_=cv)
        bout_t = const.tile([1, DQ], bf16, name="bout_t")
        nc.sync.dma_start(out=bout_t, in_=bout)

        # ---- persistent SBUF tensors ----
        xgt_pool = ctx.enter_context(tc.tile_pool(name="xgt", bufs=KC, side="left"))
        xgt = [xgt_pool.tile([P, M], bf16, name=f"xgt{j}", tag="xgt") for j in range(KC)]
        xpt_pool = ctx.enter_context(tc.tile_pool(name="xpt", bufs=KQ, side="left"))
        xpt = [xpt_pool.tile([P, NPC], bf16, name=f"xpt{j}", tag="xpt") for j in range(KQ)]
        qt_pool = ctx.enter_context(tc.tile_pool(name="qt", bufs=KI, side="left"))
        qt = [qt_pool.tile([P, NPC], bf16, name=f"qt{i}", tag="qt") for i in range(KI)]
        kt_pool = ctx.enter_context(tc.tile_pool(name="kt", bufs=KI, side="left"))
        ktl = [kt_pool.tile([P, M], bf16, name=f"kt{i}", tag="kt") for i in range(KI)]
        v_pool = ctx.enter_context(tc.tile_pool(name="vsb", bufs=MT, side="left"))
        vt = [v_pool.tile([P, H, DH + 1], bf16, name=f"v{i}", tag="v") for i in range(MT)]
        att_pool = ctx.enter_context(tc.tile_pool(name="att", bufs=KI, side="left"))
        att = [att_pool.tile([P, NPC], bf16, name=f"att{i}", tag="att") for i in range(KI)]
        oxb_pool = ctx.enter_context(tc.tile_pool(name="oxb", bufs=16, side="left"))
        oxb = [oxb_pool.tile([P, 512], bf16, name=f"oxb{i}", tag="oxb") for i in range(16)]

        wv_pool = ctx.enter_context(tc.tile_pool(name="wvp", bufs=KC, side="left"))
        wv_t = [wv_pool.tile([P, INNER], bf16, name=f"wv{i}", tag="wv") for i in range(KC)]
        wft_pool = ctx.enter_context(tc.tile_pool(name="wftp", bufs=KQ, side="left"))
        wft_t = [wft_pool.tile([P, DQ], bf16, name=f"wft{i}", tag="wft") for i in range(KQ)]

        stat_pool = ctx.enter_context(tc.tile_pool(name="stat", bufs=1, side="left"))
        gcols = stat_pool.tile([P, MT, 3], f32, name="gcols")   # (-mu, std, rstd) per mt
        pcols = stat_pool.tile([P, NT, 3], f32, name="pcols")
        corr_g = stat_pool.tile([2, M], bf16, name="corr_g")    # rows: -mu, std over m
        corr_p = stat_pool.tile([2, NPC], bf16, name="corr_p")  # rows: -mu, std over n
        rstd_g_bc = stat_pool.tile([P, M], f32, name="rstd_g_bc")
        rstd_p_bc = stat_pool.tile([P, NPC], f32, name="rstd_p_bc")

        # ---- transient pools ----
        stage = ctx.enter_context(tc.tile_pool(name="stage", bufs=2, side="right"))
        wkq_pool = ctx.enter_context(tc.tile_pool(name="wkq", bufs=2, side="right"))
        ex_pool = ctx.enter_context(tc.tile_pool(name="ex", bufs=2, side="right"))
        rc_pool = ctx.enter_context(tc.tile_pool(name="rc", bufs=1, side="right"))
        out_pool = ctx.enter_context(tc.tile_pool(name="outp", bufs=2, side="right"))
        wof_pool = ctx.enter_context(tc.tile_pool(name="wofp", bufs=KI, side="right"))

        # ============ LN stats + PE transposes ============
        def stats_tile(x_dram, i, d, cols, xT):
            """cols[:, i, :] = (-mu, std, rstd) for rows [i*128, (i+1)*128);
            also transposes the row tile into the feature-major xT tiles
            on the TensorEngine (x_blk.T @ I) -- DMA-transpose is ~10x
            slower than this."""
            x_t = stage.tile([P, d], bf16, tag="stx")
            nc.sync.dma_start(out=x_t, in_=x_dram[i * P:(i + 1) * P, :])
            nk = d // P
            for j0 in range(0, nk, 4):
                jn = min(4, nk - j0)
                tp = ch_pool.tile([P, 512], f32, tag="ch")
                for j in range(j0, j0 + jn):
                    nc.tensor.matmul(
                        tp[:, (j - j0) * P:(j - j0 + 1) * P],
                        x_t[:, j * P:(j + 1) * P], idn_t,
                    )
                for j in range(j0, j0 + jn):
                    nc.vector.tensor_copy(
                        xT[j][:, i * P:(i + 1) * P],
                        tp[:, (j - j0) * P:(j - j0 + 1) * P],
                    )
            fmax = min(nc.vector.BN_STATS_FMAX, d)
            while d % fmax:
                fmax //= 2
            nsub = d // fmax
            stats = small.tile([P, nsub, nc.vector.BN_STATS_DIM], f32, tag="stats")
            xv = x_t.rearrange("p (s f) -> p s f", s=nsub)
            for s in range(nsub):
                nc.vector.bn_stats(out=stats[:, s, :], in_=xv[:, s, :])
            mv = small.tile([P, nc.vector.BN_AGGR_DIM], f32, tag="mv")
            nc.vector.bn_aggr(out=mv, in_=stats)
            nc.vector.tensor_scalar_mul(cols[:, i, 0:1], mv[:, 0:1], -1.0)
            nc.scalar.activation(out=cols[:, i, 1:2], in_=mv[:, 1:2], func=AF.Sqrt, bias=eps_t)
            nc.vector.reciprocal(out=cols[:, i, 2:3], in_=cols[:, i, 1:2])

        def stats_flush(cols, ntile, stat_d):
            for i in range(ntile):
                # DRAM layout per tile: [3, 128] (rows -mu/std/rstd).
                dst = bass.AP(tensor=stat_d.tensor, offset=stat_d.offset + i * 3 * P,
                              ap=[[1, P], [P, 3]])
                nc.sync.dma_start(out=dst, in_=cols[:, i, :])

        # Reload stats as rows: corr (bf16 [2, n]); rstd broadcast built with
        # a K=1 ones-matmul into PSUM (partition-broadcast DMAs are slow).
        def stats_rows(stat_d, ntile, corr_bf, rstd_bc):
            corr_f = small.tile([2, ntile * P], f32, tag="scr4k", bufs=1)
            nc.sync.dma_start(
                out=corr_f,
                in_=bass.AP(tensor=stat_d.tensor, offset=stat_d.offset,
                            ap=[[P, 2], [3 * P, ntile], [1, P]]),
            )
            nc.vector.tensor_copy(corr_bf, corr_f)
            rrow = small.tile([1, ntile * P], f32, tag="scr4k", bufs=1)
            nc.sync.dma_start(
                out=rrow,
                in_=bass.AP(tensor=stat_d.tensor, offset=stat_d.offset + 2 * P,
                            ap=[[3 * P, ntile], [1, P]]),
            )
            for half in range(ntile * P // 512):
                bb = ch_pool.tile([P, 512], f32, tag="ch")
                nc.tensor.matmul(
                    bb, ones_f, rrow[0:1, half * 512:(half + 1) * 512],
                    start=True, stop=True,
                )
                nc.vector.tensor_copy(rstd_bc[:, half * 512:(half + 1) * 512], bb)

        # Prologue order: garment-side stream gates the first attention, so
        # it goes first.  The stats pipeline rides the sync queue; the
        # scalar (ACT) queue -- idle until the first exp -- carries the
        # transposes and weights in parallel.
        for i in range(MT):
            stats_tile(xg, i, DC, gcols, xgt)
        for kt in range(KC):
            nc.scalar.dma_start(out=wv_t[kt], in_=wv[kt * P:(kt + 1) * P, :])
        stats_flush(gcols, MT, statg_d)
        stats_rows(statg_d, MT, corr_g, rstd_g_bc)
        for i in range(NT):
            stats_tile(xp, i, DQ, pcols, xpt)
        stats_flush(pcols, NT, statp_d)
        stats_rows(statp_d, NT, corr_p, rstd_p_bc)
        for kt in range(KQ):
            nc.scalar.dma_start(out=wft_t[kt], in_=wft[kt * P:(kt + 1) * P, :])

        # ============ per-pair projection chains ============
        def emit_bk(p):
            wkc = wkq_pool.tile([P, KC, P], bf16, tag="wk")
            nc.sync.dma_start(
                out=wkc,
                in_=wk[:, p * P:(p + 1) * P].rearrange("(t p) c -> p t c", p=P),
            )
            for mch in range(M // 512):
                ps = ch_pool.tile([P, 512], f32, tag="ch")
                for kt in range(KC):
                    nc.tensor.matmul(
                        ps, wkc[:, kt, :], xgt[kt][:, mch * 512:(mch + 1) * 512],
                        start=(kt == 0), stop=False,
                    )
                nc.tensor.matmul(
                    ps, ck_t[:, p * P:(p + 1) * P],
                    corr_g[:, mch * 512:(mch + 1) * 512],
                    start=False, stop=True,
                )
                nc.vector.tensor_mul(
                    ktl[p][:, mch * 512:(mch + 1) * 512], ps,
                    rstd_g_bc[:, mch * 512:(mch + 1) * 512],
                )

        def emit_bq(p):
            wqc = wkq_pool.tile([P, KQ, P], bf16, tag="wq")
            nc.sync.dma_start(
                out=wqc,
                in_=wq[:, p * P:(p + 1) * P].rearrange("(t p) c -> p t c", p=P),
            )
            for nch in range(NPC // 512):
                ps = ch_pool.tile([P, 512], f32, tag="ch")
                for kt in range(KQ):
                    nc.tensor.matmul(
                        ps, wqc[:, kt, :], xpt[kt][:, nch * 512:(nch + 1) * 512],
                        start=(kt == 0), stop=False,
                    )
                nc.tensor.matmul(
                    ps, cq_t[:, p * P:(p + 1) * P],
                    corr_p[:, nch * 512:(nch + 1) * 512],
                    start=False, stop=True,
                )
                nc.vector.tensor_mul(
                    qt[p][:, nch * 512:(nch + 1) * 512], ps,
                    rstd_p_bc[:, nch * 512:(nch + 1) * 512],
                )

        def emit_bv(mt):
            nc.vector.memset(vt[mt][:, :, DH:DH + 1], 1.0)
            for ich in range(2):
                ps = ch_pool.tile([P, 512], f32, tag="ch")
                for kt in range(KC):
                    nc.tensor.matmul(
                        ps, xgt[kt][:, mt * P:(mt + 1) * P],
                        wv_t[kt][:, ich * 512:(ich + 1) * 512],
                        start=(kt == 0), stop=False,
                    )
                nc.tensor.matmul(
                    ps, corr_g[:, mt * P:(mt + 1) * P],
                    cv_t[:, ich * 512:(ich + 1) * 512],
                    start=False, stop=True,
                )
                nc.vector.tensor_scalar_mul(
                    vt[mt][:, ich * 8:(ich + 1) * 8, 0:DH],
                    ps.rearrange("p (h d) -> p h d", h=8),
                    gcols[:, mt, 2:3],
                )

        def emit_dx(ch, nt):
            ps = ch_pool.tile([P, 512], f32, tag="ch")
            for kt in range(KQ):
                nc.tensor.matmul(
                    ps, xpt[kt][:, nt * P:(nt + 1) * P],
                    wft_t[kt][:, ch * 512:(ch + 1) * 512],
                    start=(kt == 0), stop=False,
                )
            nc.tensor.matmul(
                ps, ones_row, bout_t[:, ch * 512:(ch + 1) * 512],
                start=False, stop=True,
            )
            nc.vector.tensor_copy(oxb[ch * 8 + nt], ps)

        # ============ attention ============
        def emit_attn(p, nch):
            idx = p * 2 + nch
            pa = pa_pool.tile([P, 1024], f32, tag="pa")
            for mt in range(MT):
                sg = sg_pool.tile([P, 1024], f32, tag="sg")
                nc.tensor.matmul(
                    sg[:, 0:512],
                    ktl[p][0:DH, mt * P:(mt + 1) * P],
                    qt[p][0:DH, nch * 512:(nch + 1) * 512],
                )
                nc.tensor.matmul(
                    sg[:, 512:1024],
                    ktl[p][DH:P, mt * P:(mt + 1) * P],
                    qt[p][DH:P, nch * 512:(nch + 1) * 512],
                )
                ex = ex_pool.tile([P, 1024], bf16, tag="ex")
                nc.scalar.activation(out=ex, in_=sg, func=AF.Exp)
                nc.tensor.matmul(
                    pa[0:DH + 1, 0:512], vt[mt][:, 2 * p, :], ex[:, 0:512],
                    start=(mt == 0), stop=(mt == MT - 1),
                )
                nc.tensor.matmul(
                    pa[0:DH + 1, 512:1024], vt[mt][:, 2 * p + 1, :], ex[:, 512:1024],
                    start=(mt == 0), stop=(mt == MT - 1),
                )
            # Store UNNORMALIZED att (bf16) + denominators; softmax division
            # is deferred to one batched reciprocal at the end.
            rowtmp = rc_pool.tile([1, 1024], f32, tag="rc")
            nc.vector.tensor_copy(rowtmp, pa[DH:DH + 1, :])
            nc.sync.dma_start(out=rb_d[idx:idx + 1, :], in_=rowtmp)
            nc.vector.tensor_copy(
                att[p][0:DH, nch * 512:(nch + 1) * 512], pa[0:DH, 0:512]
            )
            nc.vector.tensor_copy(
                att[p][DH:P, nch * 512:(nch + 1) * 512], pa[0:DH, 512:1024]
            )

        # ============ program ============
        emit_bk(0)
        emit_bq(0)
        for mt in range(MT):
            emit_bv(mt)
        wofc = []
        for p in range(NPAIR):
            emit_attn(p, 0)
            emit_attn(p, 1)
            if p + 1 < NPAIR:
                emit_bk(p + 1)
                emit_bq(p + 1)
            if p >= 1:
                emit_dx(0, p - 1)
                emit_dx(1, p - 1)
            if p == 5:
                for it in range(KI):
                    w_t = wof_pool.tile([P, DQ], bf16, tag="wof")
                    nc.scalar.dma_start(out=w_t, in_=wof[it * P:(it + 1) * P, :])
                    wofc.append(w_t)
        emit_dx(0, NT - 1)
        emit_dx(1, NT - 1)

        # ============ softmax normalization + output ============
        # One batched reciprocal over all denominators; broadcast across
        # partitions via a K=2 selector matmul into PSUM (no DMA).
        dn = small.tile([2 * H, 512], f32, tag="scr4k", bufs=1)
        nc.sync.dma_start(
            out=dn,
            in_=bass.AP(tensor=rb_d.tensor, offset=rb_d.offset,
                        ap=[[512, 2 * H], [1, 512]]),
        )
        nc.vector.reciprocal(out=dn, in_=dn)
        nc.sync.dma_start(
            out=bass.AP(tensor=rb2_d.tensor, offset=rb2_d.offset,
                        ap=[[512, 2 * H], [1, 512]]),
            in_=dn,
        )
        for nch in range(2):
            for p in range(NPAIR):
                idx = p * 2 + nch
                dnp = rc_pool.tile([2, 512], f32, tag="dnp", bufs=2)
                nc.sync.dma_start(
                    out=dnp,
                    in_=bass.AP(tensor=rb2_d.tensor,
                                offset=rb2_d.offset + idx * 1024,
                                ap=[[512, 2], [1, 512]]),
                )
                bcp = pa_pool.tile([P, 1024], f32, tag="pa")
                nc.tensor.matmul(
                    bcp[:, 0:512], sel, dnp,
                    start=True, stop=True,
                )
                nc.vector.tensor_mul(
                    att[p][:, nch * 512:(nch + 1) * 512],
                    att[p][:, nch * 512:(nch + 1) * 512],
                    bcp[:, 0:512],
                )
            for nt in range(nch * 4, nch * 4 + 4):
                for ch in range(2):
                    ps = ch_pool.tile([P, 512], f32, tag="ch")
                    for it in range(KI):
                        nc.tensor.matmul(
                            ps, att[it][:, nt * P:(nt + 1) * P],
                            wofc[it][:, ch * 512:(ch + 1) * 512],
                            start=(it == 0), stop=(it == KI - 1),
                        )
                    o_t = out_pool.tile([P, 512], f32, tag="o")
                    nc.vector.tensor_add(o_t, ps, oxb[ch * 8 + nt])
                    nc.sync.dma_start(
                        out=out[nt * P:(nt + 1) * P, ch * 512:(ch + 1) * 512],
                        in_=o_t,
                    )

    nc.compile()
    return nc


def get_nc():
    if "nc" not in _CACHE:
        _CACHE["nc"] = _build_nc()
    return _CACHE["nc"]


def make_in_maps(inputs):
    """Host-side folding + sharding. Returns one input dict per core."""
    bf = ml_dtypes.bfloat16
    pf_ = np.asarray(inputs["person_features"], np.float32)
    gf_ = np.asarray(inputs["garment_features"], np.float32)
    Wq = np.asarray(inputs["Wq"], np.float32)
    Wk = np.asarray(inputs["Wk"], np.float32)
    Wv = np.asarray(inputs["Wv"], np.float32)
    Wo = np.asarray(inputs["Wo"], np.float32)
    bo = np.asarray(inputs["bo"], np.float32)
    Wf = np.asarray(inputs["Wf"], np.float32)
    bff = np.asarray(inputs["bf"], np.float32)
    gq = np.asarray(inputs["gq"], np.float32)
    betaq = np.asarray(inputs["betaq"], np.float32)
    gk = np.asarray(inputs["gk"], np.float32)
    betak = np.asarray(inputs["betak"], np.float32)

    wq_f = ((gq[:, None] * Wq) * np.float32(SCALE)).astype(bf)
    bq_f = (betaq @ Wq) * np.float32(SCALE)
    wk_f = (gk[:, None] * Wk).astype(bf)
    bk_f = betak @ Wk
    wv_f = (gk[:, None] * Wv).astype(bf)
    bv_f = betak @ Wv
    wf_top = np.ascontiguousarray(Wf[:DQ])
    wf_bot = Wf[DQ:]
    wof = (Wo.astype(np.float64) @ wf_bot.astype(np.float64)).astype(np.float32)
    bout = (bo @ wf_bot + bff).astype(np.float32)

    # Correction rows: [colsum(W'); bias] per projection (colsums of the
    # bf16-rounded weights actually used on-device).
    def corr(w_b, bias):
        cs = w_b.astype(np.float64).sum(axis=0).astype(np.float32)
        return np.stack([cs, bias]).astype(bf)

    shared = {
        "wq": wq_f,
        "wk": wk_f,
        "wv": wv_f,
        "wof": wof.astype(bf),
        "wft": wf_top.astype(bf),
        "cq": corr(wq_f, bq_f),
        "ck": corr(wk_f, bk_f),
        "cv": corr(wv_f, bv_f),
        "bout": bout[None, :].astype(bf),
        "idn": np.eye(P, dtype=np.float32).astype(bf),
        "selc": np.concatenate(
            [np.repeat([[1.0, 0.0]], DH, 0), np.repeat([[0.0, 1.0]], DH, 0)]
        ).T.astype(np.float32).copy(),
    }
    in_maps = []
    for core in range(NCORES):
        b, half = divmod(core, 2)
        m = dict(shared)
        m["xp"] = np.ascontiguousarray(pf_[b, half * NPC:(half + 1) * NPC]).astype(bf)
        m["xg"] = np.ascontiguousarray(gf_[b]).astype(bf)
        in_maps.append(m)
    return in_maps


def assemble(results):
    out = np.empty((B, N, DQ), np.float32)
    for core in range(NCORES):
        b, half = divmod(core, 2)
        out[b, half * NPC:(half + 1) * NPC] = results[core]["out"]
    return out


def kernel(**inputs):
    from concourse.bass_utils import run_bass_kernel_spmd

    nc = get_nc()
    in_maps = make_in_maps(inputs)
    res = run_bass_kernel_spmd(nc, in_maps, list(range(NCORES)))
    return assemble(res.results)
